# revision 6
# baseline (speedup 1.0000x reference)
"""Trainium2 Bass kernel for DigitConvolutionalModel.

Model: x[B,784] -> reshape [B,1,28,28] -> 3x3 valid conv (1 channel)
       -> flatten [B,676] -> relu(@w1[676,128] + b1) -> @w2[128,10] + b2.

Strategy (memory-bound; per-core roofline is streaming the 25.7 MB x shard):
  * Conv is linear, so fold it into fc1 during weight prep: W_eff[784,128] =
    C @ w1 where C[784,676] is the conv-as-matmul operator. The device
    computes relu(x @ W_eff + b1) @ w2 + b2 -- one 784-contraction matmul and
    one 128-contraction matmul over the full batch.
  * Pure data parallel over 8 NeuronCores: batch dim sharded 8 x 8192, tiny
    weights replicated.
  * Sharding layout: each core's shard is laid out feature-major ([784, 8192],
    part of the host-side shard prep) so the TensorE contraction dim lands on
    SBUF partitions straight off the DMA -- no on-device transposes. Same
    bytes streamed; the PE then only does fc1+fc2. (A fully device-side
    transpose variant is kept as MODE="device_t"; it PE-transposes x tiles
    via fp32r transpose-mode matmuls and runs ~15% slower.)
  * Per 512-batch block: one 1.6 MB load [128, 6, 512] (+ a one-off upfront
    load of the 16-row feature tail for all blocks), 7 accumulating fc1
    matmuls into PSUM, bias+relu split across DVE and ACT halves into fp32r,
    then fc2 as out^T[10,512] = w2.T @ h^T (10-column stationary, near-zero
    weight-load cost). Output is stored contiguously in [block, 10, 512]
    layout and untransposed on the host during the gather step.
  * Loads alternate between the SP and ACT HWDGE rings (per-DMA issue cost
    hides under the other ring's in-flight transfer); each block's store goes
    to the ring opposite its load; constants ride SWDGE (gpsimd).
  * All matmuls in float32r (TF32, 10-bit mantissa: 1 cyc/row at N>=256 vs 4
    for fp32). End-to-end rel err vs the fp32 reference ~3e-4.
"""

import sys

sys.path.insert(0, "/opt/trn_rl_repo")

import numpy as np

import concourse.bass as bass
import concourse.bacc as bacc
import concourse.mybir as mybir
import concourse.tile as tile
from concourse.bass_utils import run_bass_kernel_spmd

N_CORES = 8
B_FULL = 65536
B_CORE = B_FULL // N_CORES  # 8192
D_IN = 784  # 28*28
KC = 112  # contraction chunk (784 = 7*112)
NCHUNK = 7
D_HID = 128
D_OUT = 10
D_OUT_PAD = 16
BLK = 512  # batch block per fc1 matmul group
SUB = 128  # batch sub-tile (partition dim)
NSUB = BLK // SUB  # 4
NBLK = B_CORE // BLK  # 16
KC6 = 128  # host-transposed variant contracts in chunks of 128 (+ a 16-row tail)
NC6 = 6
KTAIL = D_IN - NC6 * KC6  # 16

_compiled = None
MODE = "h16"  # "h16" (fp16 stream), "host_t" (fp32 stream), "device_t"
LB16 = 4  # blocks per DMA group in h16 mode (one 1.5 MB contiguous load)
NG16 = NBLK // LB16


def _round_tf32(a: np.ndarray) -> np.ndarray:
    """Round fp32 to tf32 (10 explicit mantissa bits), round-to-nearest-even."""
    i = a.astype(np.float32).view(np.uint32).astype(np.uint64)
    round_bit = (i >> 13) & 1
    i = (i + 0xFFF + round_bit) & np.uint64(0xFFFFE000)
    return i.astype(np.uint32).view(np.float32)


def _build_weff(conv_w: np.ndarray, w1: np.ndarray) -> np.ndarray:
    """W_eff[784,128]: folded conv+fc1 weights."""
    w1v = w1.astype(np.float64).reshape(26, 26, D_HID)
    acc = np.zeros((28, 28, D_HID), dtype=np.float64)
    cw = conv_w.astype(np.float64)
    for dr in range(3):
        for dc in range(3):
            acc[dr : dr + 26, dc : dc + 26, :] += cw[dr, dc] * w1v
    w_eff = acc.reshape(D_IN, D_HID).astype(np.float32)
    return _round_tf32(w_eff)


def _build_bass(xin_bufs=5, xt_bufs=4, h_bufs=3, o_bufs=3, pxt_bufs=3,
                ph_bufs=1, po_bufs=1, depth=2, repeat=1, mode="device_t"):
    if mode == "h16":
        return _build_bass_h16(repeat=repeat)
    if mode == "host_t":
        # host_t has its own tuned pool defaults; only forward repeat
        return _build_bass_host_t(repeat=repeat)
    nc = bacc.Bacc("TRN2", target_bir_lowering=False, debug=False, num_devices=1)
    f32 = mybir.dt.float32
    f32r = mybir.dt.float32r

    x_d = nc.dram_tensor("x", [B_CORE, D_IN], f32r, kind="ExternalInput").ap()
    w_d = nc.dram_tensor("w", [NCHUNK, KC, D_HID], f32r,
                         kind="ExternalInput").ap()
    b1_d = nc.dram_tensor("b1", [D_HID], f32, kind="ExternalInput").ap()
    w2_d = nc.dram_tensor("w2", [D_HID, D_OUT_PAD], f32r, kind="ExternalInput").ap()
    b2_d = nc.dram_tensor("b2", [NSUB * D_OUT_PAD], f32, kind="ExternalInput").ap()
    id_d = nc.dram_tensor("idn", [SUB, SUB], f32r, kind="ExternalInput").ap()
    out_d = nc.dram_tensor("out", [B_CORE, D_OUT_PAD], f32, kind="ExternalOutput").ap()

    # out viewed as [block, 128, sub, 16] so store order matches o_sb's (p, s, c)
    out_v = out_d.rearrange("(t s p) c -> t p s c", s=NSUB, p=SUB)
    # x viewed as [block, 128, sub, 784]: one 1.6 MB DMA per block brings in
    # all 4 batch sub-tiles, laid out [p, s, f] in SBUF
    x_v = x_d.rearrange("(t s p) c -> t p s c", s=NSUB, p=SUB)

    with tile.TileContext(nc) as tc:
        with (
            tc.tile_pool(name="const", bufs=1) as const_pool,
            tc.tile_pool(name="xin", bufs=xin_bufs) as xpool,
            tc.tile_pool(name="xt", bufs=xt_bufs) as xtpool,
            tc.tile_pool(name="h", bufs=h_bufs) as hpool,
            tc.tile_pool(name="o", bufs=o_bufs) as opool,
            tc.tile_pool(name="pxt", bufs=pxt_bufs, space="PSUM") as ps_xt,
            tc.tile_pool(name="ph", bufs=ph_bufs, space="PSUM") as ps_h,
            tc.tile_pool(name="po", bufs=po_bufs, space="PSUM") as ps_o,
        ):
            w_sb = const_pool.tile([KC, NCHUNK, D_HID], f32r)
            nc.sync.dma_start(w_sb, w_d.rearrange("c k h -> k c h"))
            b1_sb = const_pool.tile([D_HID, 1], f32)
            nc.sync.dma_start(b1_sb, b1_d.rearrange("(h o) -> h o", o=1))
            w2_sb = const_pool.tile([D_HID, D_OUT_PAD], f32r)
            nc.sync.dma_start(w2_sb, w2_d)
            id_sb = const_pool.tile([SUB, SUB], f32r)
            nc.sync.dma_start(id_sb, id_d)
            b2_sb = const_pool.tile([SUB, NSUB * D_OUT_PAD], f32)
            b2_bcast = bass.AP(
                tensor=b2_d.tensor, offset=b2_d.offset,
                ap=[[0, SUB]] + list(b2_d.ap),
            )
            nc.sync.dma_start(b2_sb, b2_bcast)

            xts = {}

            def prepare(t):
                """Load block t and transpose it to feature-major."""
                xt_sb = xtpool.tile([KC, NCHUNK, BLK], f32r)
                x_sb = xpool.tile([SUB, NSUB, D_IN], f32r)
                if t == 0:
                    # fine-grained first load so block 0's transposes start
                    # after ~1.1 us instead of waiting for the full 1.6 MB
                    for s in range(NSUB):
                        nc.sync.dma_start(x_sb[:, s, :], x_v[t, :, s, :])
                else:
                    nc.sync.dma_start(x_sb, x_v[t])
                for s in range(NSUB):
                    ps = ps_xt.tile([KC, NCHUNK * SUB], f32r)
                    for c in range(NCHUNK):
                        nc.tensor.matmul(
                            ps[:, c * SUB : (c + 1) * SUB],
                            x_sb[:, s, c * KC : (c + 1) * KC],
                            id_sb,
                            is_transpose=True,
                            start=True,
                            stop=True,
                        )
                    # copy all 7 transposed chunks to SBUF in one op;
                    # alternate DVE/ACT to split the copy load
                    dst = xt_sb[:, :, s * SUB : (s + 1) * SUB]
                    src = ps.rearrange("k (c b) -> k c b", c=NCHUNK)
                    if s % 2 == 0:
                        nc.vector.tensor_copy(dst, src)
                    else:
                        nc.scalar.copy(dst, src)
                xts[t] = xt_sb

            hs = {}

            def fc1_relu(t):
                """fc1 + bias-relu for block t; h^T parked in SBUF."""
                xt_sb = xts.pop(t)
                hps = ps_h.tile([D_HID, BLK], mybir.dt.float32)
                for c in range(NCHUNK):
                    nc.tensor.matmul(
                        hps,
                        w_sb[:, c, :],
                        xt_sb[:, c, :],
                        start=(c == 0),
                        stop=(c == NCHUNK - 1),
                    )
                h_sb = hpool.tile([D_HID, BLK], f32r)
                nc.scalar.activation(
                    h_sb, hps, mybir.ActivationFunctionType.Relu, bias=b1_sb
                )
                hs[t] = h_sb

            def consume(t):
                """fc2 + bias + store for block t."""
                h_sb = hs.pop(t)
                ops = ps_o.tile([SUB, NSUB, D_OUT_PAD], mybir.dt.float32)
                for s in range(NSUB):
                    nc.tensor.matmul(
                        ops[:, s, :],
                        h_sb[:, s * SUB : (s + 1) * SUB],
                        w2_sb,
                        start=True,
                        stop=True,
                    )
                o_sb = opool.tile([SUB, NSUB, D_OUT_PAD], mybir.dt.float32)
                nc.vector.tensor_add(
                    o_sb,
                    ops,
                    b2_sb.rearrange("p (s c) -> p s c", s=NSUB),
                )
                # stores ride the ACT HWDGE ring so they never block x loads
                # queued on the SP ring (HWDGE is FIFO per issuing engine)
                nc.scalar.dma_start(out_v[t], o_sb)

            # 3-stage software pipeline: by the time block t's fc2 is emitted,
            # its relu ran a stage earlier and block t+2's transposes keep the
            # in-order PE queue from stalling on the copy/relu chains
            for _ in range(repeat):
                if depth == 2:
                    prepare(0)
                    prepare(1)
                    fc1_relu(0)
                    for t in range(NBLK):
                        if t + 2 < NBLK:
                            prepare(t + 2)
                        if t + 1 < NBLK:
                            fc1_relu(t + 1)
                        consume(t)
                else:
                    prepare(0)
                    for t in range(NBLK):
                        if t + 1 < NBLK:
                            prepare(t + 1)
                        fc1_relu(t)
                        consume(t)

    nc.compile()
    return nc


def _build_bass_h16(repeat=1, lb=LB16, xt_bufs=3, h_bufs=8, o_bufs=8,
                    ph_bufs=4, po_bufs=3):
    """fp16 streaming variant: x arrives feature-major, fp16, pre-blocked on
    the host so each DMA group is one fully contiguous [128, lb*6*512] read
    (12 KB per partition line at lb=4). Matmuls run fp16 x fp16 -> fp32 PSUM
    (PE upconverts to FP22: 13-bit mantissa, same class as the tf32 path).
    HBM traffic per repeat is halved vs host_t: ~12.3 MB loads + 160 KB fp16
    stores. b2 is added on the host after the gather."""
    nc = bacc.Bacc("TRN2", target_bir_lowering=False, debug=False, num_devices=1)
    f32 = mybir.dt.float32
    f16 = mybir.dt.float16
    ng = NBLK // lb
    HB = BLK // 2

    xm_d = nc.dram_tensor("xm", [ng, SUB, lb * NC6 * BLK], f16,
                          kind="ExternalInput").ap()
    x6_d = nc.dram_tensor("x6", [KTAIL, B_CORE], f16, kind="ExternalInput").ap()
    w_d = nc.dram_tensor("w", [NC6, KC6, D_HID], f16, kind="ExternalInput").ap()
    w6_d = nc.dram_tensor("w6", [KTAIL, D_HID], f16, kind="ExternalInput").ap()
    b1_d = nc.dram_tensor("b1", [D_HID], f32, kind="ExternalInput").ap()
    w2_d = nc.dram_tensor("w2", [D_HID, D_OUT], f16, kind="ExternalInput").ap()
    out_d = nc.dram_tensor("out", [NBLK, D_OUT, BLK], f16,
                           kind="ExternalOutput").ap()

    with tile.TileContext(nc) as tc:
        with (
            tc.tile_pool(name="const", bufs=1) as const_pool,
            tc.tile_pool(name="xt", bufs=xt_bufs) as xtpool,
            tc.tile_pool(name="h", bufs=h_bufs) as hpool,
            tc.tile_pool(name="o", bufs=o_bufs) as opool,
            tc.tile_pool(name="ph", bufs=ph_bufs, space="PSUM") as ps_h,
            tc.tile_pool(name="po", bufs=po_bufs, space="PSUM") as ps_o,
        ):
            # constants ride SWDGE (gpsimd); both HWDGE rings are for x
            w_sb = const_pool.tile([KC6, NC6, D_HID], f16)
            nc.gpsimd.dma_start(w_sb, w_d.rearrange("c k h -> k c h"))
            w6_sb = const_pool.tile([KTAIL, D_HID], f16)
            nc.gpsimd.dma_start(w6_sb, w6_d)
            b1_sb = const_pool.tile([D_HID, 1], f32)
            nc.gpsimd.dma_start(b1_sb, b1_d.rearrange("(h o) -> h o", o=1))
            w2_sb = const_pool.tile([D_HID, D_OUT], f16)
            nc.gpsimd.dma_start(w2_sb, w2_d)
            # 16-row feature tail for ALL blocks, resident across repeats
            x6_sb = const_pool.tile([KTAIL, B_CORE], f16)
            nc.gpsimd.dma_start(x6_sb, x6_d)

            for r in range(repeat):
                for g in range(ng):
                    eng = nc.sync if g % 2 == 0 else nc.scalar
                    seng = nc.scalar if g % 2 == 0 else nc.sync
                    xg = xtpool.tile([SUB, lb, NC6, BLK], f16)
                    eng.dma_start(
                        xg, xm_d[g].rearrange("k (l c b) -> k l c b",
                                              l=lb, c=NC6)
                    )
                    for l in range(lb):
                        t = g * lb + l
                        hps = ps_h.tile([D_HID, BLK], f32)
                        for c in range(NC6):
                            nc.tensor.matmul(
                                hps, w_sb[:, c, :], xg[:, l, c, :],
                                start=(c == 0), stop=False,
                            )
                        nc.tensor.matmul(
                            hps, w6_sb, x6_sb[:, t * BLK : (t + 1) * BLK],
                            start=False, stop=True,
                        )
                        # bias+relu split DVE/ACT so the halves run in parallel
                        h_sb = hpool.tile([D_HID, BLK], f16)
                        nc.vector.tensor_scalar(
                            h_sb[:, :HB], hps[:, :HB], b1_sb, 0.0,
                            mybir.AluOpType.add, mybir.AluOpType.max,
                        )
                        nc.scalar.activation(
                            h_sb[:, HB:], hps[:, HB:],
                            mybir.ActivationFunctionType.Relu, bias=b1_sb,
                        )
                        # fc2 transposed: out^T[10,512] = w2.T @ h^T
                        ops = ps_o.tile([D_OUT, BLK], f32)
                        nc.tensor.matmul(ops, w2_sb, h_sb, start=True,
                                         stop=True)
                        o_sb = opool.tile([D_OUT, BLK], f16)
                        nc.vector.tensor_copy(o_sb, ops)
                        seng.dma_start(out_d[t], o_sb)

    nc.compile()
    return nc


def _get_compiled():
    global _compiled
    if _compiled is None:
        _compiled = _build_bass(mode=MODE)
    return _compiled


def _make_in_maps(x, conv_w, w1, b1, w2, b2):
    w_eff = _build_weff(conv_w, w1)
    w2p = np.zeros((D_HID, D_OUT_PAD), dtype=np.float32)
    w2p[:, :D_OUT] = _round_tf32(w2.astype(np.float32))
    b2p = np.zeros(D_OUT_PAD, dtype=np.float32)
    b2p[:D_OUT] = b2
    b2t = np.tile(b2p, NSUB)
    b1f = np.asarray(b1, dtype=np.float32)

    xs = np.asarray(x, dtype=np.float32).reshape(N_CORES, B_CORE, D_IN)
    maps = []
    for i in range(N_CORES):
        if MODE == "h16":
            x16 = xs[i].astype(np.float16)
            xm = np.ascontiguousarray(
                x16[:, : NC6 * KC6]
                .reshape(NG16, LB16, BLK, NC6, KC6)
                .transpose(0, 4, 1, 3, 2)
            ).reshape(NG16, KC6, LB16 * NC6 * BLK)
            m = {
                "xm": xm,
                "x6": np.ascontiguousarray(x16[:, NC6 * KC6 :].T),
                "w": np.ascontiguousarray(
                    w_eff[: NC6 * KC6].reshape(NC6, KC6, D_HID)
                ).astype(np.float16),
                "w6": w_eff[NC6 * KC6 :].astype(np.float16),
                "b1": b1f,
                "w2": np.asarray(w2, dtype=np.float16),
            }
        elif MODE == "host_t":
            m = {
                "xt": np.ascontiguousarray(xs[i].T),
                "w": np.ascontiguousarray(
                    w_eff[: NC6 * KC6].reshape(NC6, KC6, D_HID)
                ),
                "w6": np.ascontiguousarray(w_eff[NC6 * KC6 :]),
                "b1": b1f,
                "w2": _round_tf32(np.asarray(w2, dtype=np.float32)),
                "b2": np.asarray(b2, dtype=np.float32),
            }
        else:
            m = {
                "x": np.ascontiguousarray(xs[i]),
                "w": w_eff.reshape(NCHUNK, KC, D_HID),
                "b1": b1f,
                "w2": w2p,
                "b2": b2t,
                "idn": np.eye(SUB, dtype=np.float32),
            }
        maps.append(m)
    return maps


def kernel(x, conv_w, w1, b1, w2, b2, **run_kwargs):
    nc = _get_compiled()
    in_maps = _make_in_maps(x, conv_w, w1, b1, w2, b2)
    res = run_bass_kernel_spmd(nc, in_maps, core_ids=list(range(N_CORES)),
                               **run_kwargs)
    if MODE == "h16":
        # device layout [NBLK, 10, 512] fp16 -> [B_CORE, 10] fp32 (+ b2)
        b2f = np.asarray(b2, dtype=np.float32)
        parts = [
            r["out"].transpose(0, 2, 1).reshape(B_CORE, D_OUT)
            .astype(np.float32) + b2f[None, :]
            for r in res.results
        ]
    elif MODE == "host_t":
        # device layout [NBLK, 10, 512] -> [B_CORE, 10]
        parts = [
            r["out"].transpose(0, 2, 1).reshape(B_CORE, D_OUT)
            for r in res.results
        ]
    else:
        parts = [r["out"][:, :D_OUT] for r in res.results]
    out = np.concatenate(parts, axis=0)
    if run_kwargs:
        return out, res
    return out



def _build_bass_host_t(xt_bufs=5, h_bufs=8, o_bufs=8, ph_bufs=4, po_bufs=3,
                       repeat=1, fc2_mode="transposed", defer_fc2=False,
                       load2=True, load_blocks=None):
    """Variant that receives x already feature-major ([784, 8192] per core):
    no on-device transposes; fc1 streams xT tiles straight from HBM.

    Output is written in the on-chip-natural layout ([NBLK, 10, 512] for the
    transposed fc2, [NBLK, 128, NSUB, 16] for batch-major fc2) with contiguous
    stores; the host unscrambles to [8192, 10] in the gather step."""
    nc = bacc.Bacc("TRN2", target_bir_lowering=False, debug=False, num_devices=1)
    f32 = mybir.dt.float32
    f32r = mybir.dt.float32r
    HB = BLK // 2  # fc1/relu column half

    xt_d = nc.dram_tensor("xt", [D_IN, B_CORE], f32r, kind="ExternalInput").ap()
    w_d = nc.dram_tensor("w", [NC6, KC6, D_HID], f32r, kind="ExternalInput").ap()
    w6_d = nc.dram_tensor("w6", [KTAIL, D_HID], f32r, kind="ExternalInput").ap()
    b1_d = nc.dram_tensor("b1", [D_HID], f32, kind="ExternalInput").ap()
    w2_d = nc.dram_tensor("w2", [D_HID, D_OUT], f32r, kind="ExternalInput").ap()
    b2_d = nc.dram_tensor("b2", [D_OUT], f32, kind="ExternalInput").ap()
    if fc2_mode == "transposed":
        out_d = nc.dram_tensor("out", [NBLK, D_OUT, BLK], f32,
                               kind="ExternalOutput").ap()
    else:
        out_d = nc.dram_tensor("out", [NBLK, SUB, NSUB, D_OUT_PAD], f32,
                               kind="ExternalOutput").ap()

    xt_main = xt_d[0 : NC6 * KC6, :].rearrange("(c k) b -> k c b", k=KC6)
    xt_tail = xt_d[NC6 * KC6 :, :]

    with tile.TileContext(nc) as tc:
        with (
            tc.tile_pool(name="const", bufs=1) as const_pool,
            tc.tile_pool(name="xt", bufs=xt_bufs) as xtpool,
            tc.tile_pool(name="h", bufs=h_bufs) as hpool,
            tc.tile_pool(name="o", bufs=o_bufs) as opool,
            tc.tile_pool(name="ph", bufs=ph_bufs, space="PSUM") as ps_h,
            tc.tile_pool(name="po", bufs=po_bufs, space="PSUM") as ps_o,
        ):
            # constants ride SWDGE (gpsimd); both HWDGE rings are reserved
            # for the x stream
            w_sb = const_pool.tile([KC6, NC6, D_HID], f32r)
            nc.gpsimd.dma_start(w_sb, w_d.rearrange("c k h -> k c h"))
            w6_sb = const_pool.tile([KTAIL, D_HID], f32r)
            nc.gpsimd.dma_start(w6_sb, w6_d)
            b1_sb = const_pool.tile([D_HID, 1], f32)
            nc.gpsimd.dma_start(b1_sb, b1_d.rearrange("(h o) -> h o", o=1))
            if fc2_mode == "transposed":
                w2_sb = const_pool.tile([D_HID, D_OUT], f32r)
                nc.gpsimd.dma_start(w2_sb, w2_d)
                b2_sb = const_pool.tile([D_OUT, 1], f32)
                nc.gpsimd.dma_start(b2_sb, b2_d.rearrange("(c o) -> c o", o=1))
            else:
                # cols 10..15 stay garbage; host strips them
                w2_sb = const_pool.tile([D_HID, D_OUT_PAD], f32r)
                nc.gpsimd.dma_start(w2_sb[:, :D_OUT], w2_d)
                b2_sb = const_pool.tile([SUB, NSUB, D_OUT_PAD], f32)
                b2_bcast = bass.AP(
                    tensor=b2_d.tensor, offset=b2_d.offset,
                    ap=[[0, SUB], [0, NSUB]] + list(b2_d.ap),
                )
                nc.gpsimd.dma_start(b2_sb[:, :, :D_OUT], b2_bcast)
            # the 16-row feature tail for ALL blocks in one upfront DMA
            xt6_all = const_pool.tile([KTAIL, B_CORE], f32r)
            nc.gpsimd.dma_start(xt6_all, xt_tail)

            hs = {}

            def fc2_store(t):
                h_sb = hs.pop(t)
                seng = nc.scalar if t % 2 == 0 else nc.sync
                if fc2_mode == "transposed":
                    # fc2 transposed: out^T[10, 512] = w2.T @ h^T -- one
                    # matmul with a 10-column stationary (near-zero LDW);
                    # host untransposes the [NBLK, 10, 512] output
                    ops = ps_o.tile([D_OUT, BLK], f32)
                    nc.tensor.matmul(ops, w2_sb, h_sb, start=True, stop=True)
                    o_sb = opool.tile([D_OUT, BLK], f32)
                    nc.vector.tensor_scalar(
                        o_sb, ops, b2_sb, None, mybir.AluOpType.add
                    )
                    seng.dma_start(out_d[t], o_sb)
                else:
                    ops = ps_o.tile([SUB, NSUB, D_OUT_PAD], f32)
                    for s in range(NSUB):
                        nc.tensor.matmul(
                            ops[:, s, :],
                            h_sb[:, s * SUB : (s + 1) * SUB],
                            w2_sb, start=True, stop=True,
                        )
                    o_sb = opool.tile([SUB, NSUB, D_OUT_PAD], f32)
                    nc.vector.tensor_add(o_sb, ops, b2_sb)
                    seng.dma_start(out_d[t], o_sb)

            xt2s = {}

            for r in range(repeat):
                for t in range(NBLK):
                    # alternate the two HWDGE rings (SP / ACT) per block
                    eng = nc.sync if t % 2 == 0 else nc.scalar
                    LB = load_blocks or (2 if load2 else 1)
                    if LB > 1:
                        # one DMA covers LB blocks: longer contiguous runs
                        # and 1/LB the per-ring issue count
                        if t % LB == 0:
                            xt2 = xtpool.tile([KC6, NC6, LB * BLK], f32r)
                            if r == 0 and t == 0:
                                # chunk-granular first block so fc1(0) starts
                                # ~1 us in instead of after the full group
                                for c in range(NC6):
                                    eng.dma_start(
                                        xt2[:, c, :BLK],
                                        xt_main[:, c, 0:BLK],
                                    )
                                eng.dma_start(
                                    xt2[:, :, BLK : LB * BLK],
                                    xt_main[:, :, BLK : LB * BLK],
                                )
                            else:
                                eng.dma_start(
                                    xt2,
                                    xt_main[:, :, t * BLK : (t + LB) * BLK],
                                )
                            xt2s[t] = xt2
                        xt2 = xt2s[t - (t % LB)]
                        base = (t % LB) * BLK
                        xt_sb = xt2[:, :, base : base + BLK]
                    else:
                        xt_sb = xtpool.tile([KC6, NC6, BLK], f32r)
                        if r == 0 and t == 0:
                            # chunk-granular so the first fc1 starts ~1 us in
                            for c in range(NC6):
                                eng.dma_start(
                                    xt_sb[:, c, :],
                                    xt_main[:, c, t * BLK : (t + 1) * BLK],
                                )
                        else:
                            eng.dma_start(
                                xt_sb, xt_main[:, :, t * BLK : (t + 1) * BLK]
                            )
                    xt6_sb = xt6_all[:, t * BLK : (t + 1) * BLK]

                    # fc1 at full N=512 (weight loads are the serial cost on
                    # PE -- keep matmul count minimal); relu+bias on DVE
                    hps = ps_h.tile([D_HID, BLK], f32)
                    h_sb = hpool.tile([D_HID, BLK], f32r)
                    for c in range(NC6):
                        nc.tensor.matmul(
                            hps, w_sb[:, c, :], xt_sb[:, c, :],
                            start=(c == 0), stop=False,
                        )
                    nc.tensor.matmul(hps, w6_sb, xt6_sb, start=False, stop=True)
                    # bias+relu split across DVE and ACT so the two halves
                    # run in parallel off the fc1->fc2 critical path
                    nc.vector.tensor_scalar(
                        h_sb[:, :HB], hps[:, :HB], b1_sb, 0.0,
                        mybir.AluOpType.add, mybir.AluOpType.max,
                    )
                    nc.scalar.activation(
                        h_sb[:, HB:], hps[:, HB:],
                        mybir.ActivationFunctionType.Relu, bias=b1_sb,
                    )
                    hs[t] = h_sb

                    if defer_fc2:
                        # emit fc2(t-1) after fc1(t): by then relu(t-1) is
                        # long done, so the in-order PE queue never stalls
                        # waiting on the relu halves
                        if t > 0:
                            fc2_store(t - 1)
                        if t == NBLK - 1:
                            fc2_store(t)
                    else:
                        fc2_store(t)

    nc.compile()
    return nc


def _build_bass_dmaonly(repeat=1, nbufs=6, ring2=True, stage=0):
    """Incremental kernel for HW stage-cost bisection.

    stage 0: loads only; 1: +fc1; 2: +relu; 3: +fc2/add; 4: +stores."""
    nc = bacc.Bacc("TRN2", target_bir_lowering=False, debug=False, num_devices=1)
    f32 = mybir.dt.float32
    f32r = mybir.dt.float32r
    xt_d = nc.dram_tensor("xt", [D_IN, B_CORE], f32r, kind="ExternalInput").ap()
    w_d = nc.dram_tensor("w", [NC6, KC6, D_HID], f32r, kind="ExternalInput").ap()
    w6_d = nc.dram_tensor("w6", [KTAIL, D_HID], f32r, kind="ExternalInput").ap()
    b1_d = nc.dram_tensor("b1", [D_HID], f32, kind="ExternalInput").ap()
    w2_d = nc.dram_tensor("w2", [D_HID, D_OUT_PAD], f32r, kind="ExternalInput").ap()
    b2_d = nc.dram_tensor("b2", [NSUB * D_OUT_PAD], f32, kind="ExternalInput").ap()
    out_d = nc.dram_tensor("out", [B_CORE, D_OUT_PAD], f32,
                           kind="ExternalOutput").ap()
    out_v = out_d.rearrange("(t s p) c -> t p s c", s=NSUB, p=SUB)
    xt_main = xt_d[0 : NC6 * KC6, :].rearrange("(c k) b -> k c b", k=KC6)
    xt_tail = xt_d[NC6 * KC6 :, :]
    with tile.TileContext(nc) as tc:
        with (
            tc.tile_pool(name="const", bufs=1) as const_pool,
            tc.tile_pool(name="xt", bufs=nbufs) as xtpool,
            tc.tile_pool(name="h", bufs=4) as hpool,
            tc.tile_pool(name="o", bufs=4) as opool,
            tc.tile_pool(name="ph", bufs=2, space="PSUM") as ps_h,
            tc.tile_pool(name="po", bufs=2, space="PSUM") as ps_o,
        ):
            w_sb = const_pool.tile([KC6, NC6, D_HID], f32r)
            nc.gpsimd.dma_start(w_sb, w_d.rearrange("c k h -> k c h"))
            w6_sb = const_pool.tile([KTAIL, D_HID], f32r)
            nc.gpsimd.dma_start(w6_sb, w6_d)
            b1_sb = const_pool.tile([D_HID, 1], f32)
            nc.gpsimd.dma_start(b1_sb, b1_d.rearrange("(h o) -> h o", o=1))
            w2_sb = const_pool.tile([D_HID, D_OUT_PAD], f32r)
            nc.gpsimd.dma_start(w2_sb, w2_d)
            b2_sb = const_pool.tile([SUB, NSUB * D_OUT_PAD], f32)
            b2_bcast = bass.AP(
                tensor=b2_d.tensor, offset=b2_d.offset,
                ap=[[0, SUB]] + list(b2_d.ap),
            )
            nc.gpsimd.dma_start(b2_sb, b2_bcast)
            xt6_all = const_pool.tile([KTAIL, B_CORE], f32r)
            nc.gpsimd.dma_start(xt6_all, xt_tail)
            o_dummy = const_pool.tile([SUB, D_OUT_PAD], f32)
            nc.gpsimd.memset(o_dummy, 0.0)

            for _ in range(repeat):
                for t in range(NBLK):
                    eng = nc.sync if (t % 2 == 0 or not ring2) else nc.scalar
                    xt_sb = xtpool.tile([KC6, NC6, BLK], f32r)
                    eng.dma_start(
                        xt_sb, xt_main[:, :, t * BLK : (t + 1) * BLK]
                    )
                    if stage < 1:
                        continue
                    hps = ps_h.tile([D_HID, BLK], f32)
                    for c in range(NC6):
                        nc.tensor.matmul(
                            hps, w_sb[:, c, :], xt_sb[:, c, :],
                            start=(c == 0), stop=False,
                        )
                    nc.tensor.matmul(
                        hps, w6_sb, xt6_all[:, t * BLK : (t + 1) * BLK],
                        start=False, stop=True,
                    )
                    if stage < 2:
                        continue
                    h_sb = hpool.tile([D_HID, BLK], f32r)
                    nc.vector.tensor_scalar(
                        h_sb, hps, b1_sb, 0.0,
                        mybir.AluOpType.add, mybir.AluOpType.max,
                    )
                    if stage < 3:
                        continue
                    ops = ps_o.tile([SUB, NSUB, D_OUT_PAD], f32)
                    for s in range(NSUB):
                        nc.tensor.matmul(
                            ops[:, s, :],
                            h_sb[:, s * SUB : (s + 1) * SUB],
                            w2_sb, start=True, stop=True,
                        )
                    o_sb = opool.tile([SUB, NSUB, D_OUT_PAD], f32)
                    nc.vector.tensor_add(
                        o_sb, ops, b2_sb.rearrange("p (s c) -> p s c", s=NSUB)
                    )
                    if stage < 4:
                        continue
                    seng = nc.scalar if t % 2 == 0 else nc.sync
                    seng.dma_start(out_v[t], o_sb)
            nc.sync.dma_start(out_d[0:SUB, :], o_dummy)
    nc.compile()
    return nc



# revision 8
# speedup vs baseline: 1.0562x; 1.0562x over previous
"""Trainium2 Bass kernel for DigitConvolutionalModel.

Model: x[B,784] -> reshape [B,1,28,28] -> 3x3 valid conv (1 channel)
       -> flatten [B,676] -> relu(@w1[676,128] + b1) -> @w2[128,10] + b2.

Strategy (memory-bound; per-core roofline is streaming the 25.7 MB x shard):
  * Conv is linear, so fold it into fc1 during weight prep: W_eff[784,128] =
    C @ w1 where C[784,676] is the conv-as-matmul operator. The device
    computes relu(x @ W_eff + b1) @ w2 + b2 -- one 784-contraction matmul and
    one 128-contraction matmul over the full batch.
  * Pure data parallel over 8 NeuronCores: batch dim sharded 8 x 8192, tiny
    weights replicated.
  * Sharding layout: each core's shard is laid out feature-major ([784, 8192],
    part of the host-side shard prep) so the TensorE contraction dim lands on
    SBUF partitions straight off the DMA -- no on-device transposes. Same
    bytes streamed; the PE then only does fc1+fc2. (A fully device-side
    transpose variant is kept as MODE="device_t"; it PE-transposes x tiles
    via fp32r transpose-mode matmuls and runs ~15% slower.)
  * Per 512-batch block: one 1.6 MB load [128, 6, 512] (+ a one-off upfront
    load of the 16-row feature tail for all blocks), 7 accumulating fc1
    matmuls into PSUM, bias+relu split across DVE and ACT halves into fp32r,
    then fc2 as out^T[10,512] = w2.T @ h^T (10-column stationary, near-zero
    weight-load cost). Output is stored contiguously in [block, 10, 512]
    layout and untransposed on the host during the gather step.
  * Loads alternate between the SP and ACT HWDGE rings (per-DMA issue cost
    hides under the other ring's in-flight transfer); each block's store goes
    to the ring opposite its load; constants ride SWDGE (gpsimd).
  * All matmuls in float32r (TF32, 10-bit mantissa: 1 cyc/row at N>=256 vs 4
    for fp32). End-to-end rel err vs the fp32 reference ~3e-4.
"""

import sys

sys.path.insert(0, "/opt/trn_rl_repo")

import numpy as np

import concourse.bass as bass
import concourse.bacc as bacc
import concourse.mybir as mybir
import concourse.tile as tile
from concourse.bass_utils import run_bass_kernel_spmd

N_CORES = 8
B_FULL = 65536
B_CORE = B_FULL // N_CORES  # 8192
D_IN = 784  # 28*28
KC = 112  # contraction chunk (784 = 7*112)
NCHUNK = 7
D_HID = 128
D_OUT = 10
D_OUT_PAD = 16
BLK = 512  # batch block per fc1 matmul group
SUB = 128  # batch sub-tile (partition dim)
NSUB = BLK // SUB  # 4
NBLK = B_CORE // BLK  # 16
KC6 = 128  # host-transposed variant contracts in chunks of 128 (+ a 16-row tail)
NC6 = 6
KTAIL = D_IN - NC6 * KC6  # 16

_compiled = None
MODE = "h16"  # "h16" (fp16 stream), "host_t" (fp32 stream), "device_t"
LB16 = 4  # blocks per DMA group in h16 mode (one 1.5 MB contiguous load)
NG16 = NBLK // LB16


def _round_tf32(a: np.ndarray) -> np.ndarray:
    """Round fp32 to tf32 (10 explicit mantissa bits), round-to-nearest-even."""
    i = a.astype(np.float32).view(np.uint32).astype(np.uint64)
    round_bit = (i >> 13) & 1
    i = (i + 0xFFF + round_bit) & np.uint64(0xFFFFE000)
    return i.astype(np.uint32).view(np.float32)


def _build_weff(conv_w: np.ndarray, w1: np.ndarray) -> np.ndarray:
    """W_eff[784,128]: folded conv+fc1 weights."""
    w1v = w1.astype(np.float64).reshape(26, 26, D_HID)
    acc = np.zeros((28, 28, D_HID), dtype=np.float64)
    cw = conv_w.astype(np.float64)
    for dr in range(3):
        for dc in range(3):
            acc[dr : dr + 26, dc : dc + 26, :] += cw[dr, dc] * w1v
    w_eff = acc.reshape(D_IN, D_HID).astype(np.float32)
    return _round_tf32(w_eff)


def _build_bass(xin_bufs=5, xt_bufs=4, h_bufs=3, o_bufs=3, pxt_bufs=3,
                ph_bufs=1, po_bufs=1, depth=2, repeat=1, mode="device_t"):
    if mode == "h16":
        return _build_bass_h16(repeat=repeat)
    if mode == "host_t":
        # host_t has its own tuned pool defaults; only forward repeat
        return _build_bass_host_t(repeat=repeat)
    nc = bacc.Bacc("TRN2", target_bir_lowering=False, debug=False, num_devices=1)
    f32 = mybir.dt.float32
    f32r = mybir.dt.float32r

    x_d = nc.dram_tensor("x", [B_CORE, D_IN], f32r, kind="ExternalInput").ap()
    w_d = nc.dram_tensor("w", [NCHUNK, KC, D_HID], f32r,
                         kind="ExternalInput").ap()
    b1_d = nc.dram_tensor("b1", [D_HID], f32, kind="ExternalInput").ap()
    w2_d = nc.dram_tensor("w2", [D_HID, D_OUT_PAD], f32r, kind="ExternalInput").ap()
    b2_d = nc.dram_tensor("b2", [NSUB * D_OUT_PAD], f32, kind="ExternalInput").ap()
    id_d = nc.dram_tensor("idn", [SUB, SUB], f32r, kind="ExternalInput").ap()
    out_d = nc.dram_tensor("out", [B_CORE, D_OUT_PAD], f32, kind="ExternalOutput").ap()

    # out viewed as [block, 128, sub, 16] so store order matches o_sb's (p, s, c)
    out_v = out_d.rearrange("(t s p) c -> t p s c", s=NSUB, p=SUB)
    # x viewed as [block, 128, sub, 784]: one 1.6 MB DMA per block brings in
    # all 4 batch sub-tiles, laid out [p, s, f] in SBUF
    x_v = x_d.rearrange("(t s p) c -> t p s c", s=NSUB, p=SUB)

    with tile.TileContext(nc) as tc:
        with (
            tc.tile_pool(name="const", bufs=1) as const_pool,
            tc.tile_pool(name="xin", bufs=xin_bufs) as xpool,
            tc.tile_pool(name="xt", bufs=xt_bufs) as xtpool,
            tc.tile_pool(name="h", bufs=h_bufs) as hpool,
            tc.tile_pool(name="o", bufs=o_bufs) as opool,
            tc.tile_pool(name="pxt", bufs=pxt_bufs, space="PSUM") as ps_xt,
            tc.tile_pool(name="ph", bufs=ph_bufs, space="PSUM") as ps_h,
            tc.tile_pool(name="po", bufs=po_bufs, space="PSUM") as ps_o,
        ):
            w_sb = const_pool.tile([KC, NCHUNK, D_HID], f32r)
            nc.sync.dma_start(w_sb, w_d.rearrange("c k h -> k c h"))
            b1_sb = const_pool.tile([D_HID, 1], f32)
            nc.sync.dma_start(b1_sb, b1_d.rearrange("(h o) -> h o", o=1))
            w2_sb = const_pool.tile([D_HID, D_OUT_PAD], f32r)
            nc.sync.dma_start(w2_sb, w2_d)
            id_sb = const_pool.tile([SUB, SUB], f32r)
            nc.sync.dma_start(id_sb, id_d)
            b2_sb = const_pool.tile([SUB, NSUB * D_OUT_PAD], f32)
            b2_bcast = bass.AP(
                tensor=b2_d.tensor, offset=b2_d.offset,
                ap=[[0, SUB]] + list(b2_d.ap),
            )
            nc.sync.dma_start(b2_sb, b2_bcast)

            xts = {}

            def prepare(t):
                """Load block t and transpose it to feature-major."""
                xt_sb = xtpool.tile([KC, NCHUNK, BLK], f32r)
                x_sb = xpool.tile([SUB, NSUB, D_IN], f32r)
                if t == 0:
                    # fine-grained first load so block 0's transposes start
                    # after ~1.1 us instead of waiting for the full 1.6 MB
                    for s in range(NSUB):
                        nc.sync.dma_start(x_sb[:, s, :], x_v[t, :, s, :])
                else:
                    nc.sync.dma_start(x_sb, x_v[t])
                for s in range(NSUB):
                    ps = ps_xt.tile([KC, NCHUNK * SUB], f32r)
                    for c in range(NCHUNK):
                        nc.tensor.matmul(
                            ps[:, c * SUB : (c + 1) * SUB],
                            x_sb[:, s, c * KC : (c + 1) * KC],
                            id_sb,
                            is_transpose=True,
                            start=True,
                            stop=True,
                        )
                    # copy all 7 transposed chunks to SBUF in one op;
                    # alternate DVE/ACT to split the copy load
                    dst = xt_sb[:, :, s * SUB : (s + 1) * SUB]
                    src = ps.rearrange("k (c b) -> k c b", c=NCHUNK)
                    if s % 2 == 0:
                        nc.vector.tensor_copy(dst, src)
                    else:
                        nc.scalar.copy(dst, src)
                xts[t] = xt_sb

            hs = {}

            def fc1_relu(t):
                """fc1 + bias-relu for block t; h^T parked in SBUF."""
                xt_sb = xts.pop(t)
                hps = ps_h.tile([D_HID, BLK], mybir.dt.float32)
                for c in range(NCHUNK):
                    nc.tensor.matmul(
                        hps,
                        w_sb[:, c, :],
                        xt_sb[:, c, :],
                        start=(c == 0),
                        stop=(c == NCHUNK - 1),
                    )
                h_sb = hpool.tile([D_HID, BLK], f32r)
                nc.scalar.activation(
                    h_sb, hps, mybir.ActivationFunctionType.Relu, bias=b1_sb
                )
                hs[t] = h_sb

            def consume(t):
                """fc2 + bias + store for block t."""
                h_sb = hs.pop(t)
                ops = ps_o.tile([SUB, NSUB, D_OUT_PAD], mybir.dt.float32)
                for s in range(NSUB):
                    nc.tensor.matmul(
                        ops[:, s, :],
                        h_sb[:, s * SUB : (s + 1) * SUB],
                        w2_sb,
                        start=True,
                        stop=True,
                    )
                o_sb = opool.tile([SUB, NSUB, D_OUT_PAD], mybir.dt.float32)
                nc.vector.tensor_add(
                    o_sb,
                    ops,
                    b2_sb.rearrange("p (s c) -> p s c", s=NSUB),
                )
                # stores ride the ACT HWDGE ring so they never block x loads
                # queued on the SP ring (HWDGE is FIFO per issuing engine)
                nc.scalar.dma_start(out_v[t], o_sb)

            # 3-stage software pipeline: by the time block t's fc2 is emitted,
            # its relu ran a stage earlier and block t+2's transposes keep the
            # in-order PE queue from stalling on the copy/relu chains
            for _ in range(repeat):
                if depth == 2:
                    prepare(0)
                    prepare(1)
                    fc1_relu(0)
                    for t in range(NBLK):
                        if t + 2 < NBLK:
                            prepare(t + 2)
                        if t + 1 < NBLK:
                            fc1_relu(t + 1)
                        consume(t)
                else:
                    prepare(0)
                    for t in range(NBLK):
                        if t + 1 < NBLK:
                            prepare(t + 1)
                        fc1_relu(t)
                        consume(t)

    nc.compile()
    return nc


def _build_bass_h16(repeat=1, lb=LB16, xt_bufs=3, h_bufs=8, o_bufs=8,
                    ph_bufs=4, po_bufs=3, store_eng="hwdge", defer=False,
                    stage=4):
    """fp16 streaming variant: x arrives feature-major, fp16, pre-blocked on
    the host so each DMA group is one fully contiguous [128, lb*6*512] read
    (12 KB per partition line at lb=4). Matmuls run fp16 x fp16 -> fp32 PSUM
    (PE upconverts to FP22: 13-bit mantissa, same class as the tf32 path).
    HBM traffic per repeat is halved vs host_t: ~12.3 MB loads + 160 KB fp16
    stores. b2 is added on the host after the gather."""
    nc = bacc.Bacc("TRN2", target_bir_lowering=False, debug=False, num_devices=1)
    f32 = mybir.dt.float32
    f16 = mybir.dt.float16
    ng = NBLK // lb
    HB = BLK // 2

    xm_d = nc.dram_tensor("xm", [ng, SUB, lb * NC6 * BLK], f16,
                          kind="ExternalInput").ap()
    x6_d = nc.dram_tensor("x6", [KTAIL, B_CORE], f16, kind="ExternalInput").ap()
    w_d = nc.dram_tensor("w", [NC6, KC6, D_HID], f16, kind="ExternalInput").ap()
    w6_d = nc.dram_tensor("w6", [KTAIL, D_HID], f16, kind="ExternalInput").ap()
    b1_d = nc.dram_tensor("b1", [D_HID], f32, kind="ExternalInput").ap()
    w2_d = nc.dram_tensor("w2", [D_HID, D_OUT], f16, kind="ExternalInput").ap()
    out_d = nc.dram_tensor("out", [NBLK, D_OUT, BLK], f16,
                           kind="ExternalOutput").ap()

    with tile.TileContext(nc) as tc:
        with (
            tc.tile_pool(name="const", bufs=1) as const_pool,
            tc.tile_pool(name="xt", bufs=xt_bufs) as xtpool,
            tc.tile_pool(name="h", bufs=h_bufs) as hpool,
            tc.tile_pool(name="o", bufs=o_bufs) as opool,
            tc.tile_pool(name="ph", bufs=ph_bufs, space="PSUM") as ps_h,
            tc.tile_pool(name="po", bufs=po_bufs, space="PSUM") as ps_o,
        ):
            # constants ride SWDGE (gpsimd); both HWDGE rings are for x
            w_sb = const_pool.tile([KC6, NC6, D_HID], f16)
            nc.gpsimd.dma_start(w_sb, w_d.rearrange("c k h -> k c h"))
            w6_sb = const_pool.tile([KTAIL, D_HID], f16)
            nc.gpsimd.dma_start(w6_sb, w6_d)
            b1_sb = const_pool.tile([D_HID, 1], f32)
            nc.gpsimd.dma_start(b1_sb, b1_d.rearrange("(h o) -> h o", o=1))
            w2_sb = const_pool.tile([D_HID, D_OUT], f16)
            nc.gpsimd.dma_start(w2_sb, w2_d)
            # 16-row feature tail for ALL blocks, resident across repeats
            x6_sb = const_pool.tile([KTAIL, B_CORE], f16)
            nc.gpsimd.dma_start(x6_sb, x6_d)
            o_dummy = None
            if stage < 4:
                o_dummy = const_pool.tile([D_OUT, BLK], f16)
                nc.gpsimd.memset(o_dummy, 0.0)

            def fc2_store(t, h_sb, g):
                seng = nc.scalar if g % 2 == 0 else nc.sync
                ops = ps_o.tile([D_OUT, BLK], f32)
                nc.tensor.matmul(ops, w2_sb, h_sb, start=True, stop=True)
                if stage < 4:
                    return
                o_sb = opool.tile([D_OUT, BLK], f16)
                nc.vector.tensor_copy(o_sb, ops)
                if store_eng == "gpsimd":
                    nc.gpsimd.dma_start(out_d[t], o_sb)
                else:
                    seng.dma_start(out_d[t], o_sb)

            pend = None  # (t, h_sb, g) awaiting fc2 when defer=True
            for r in range(repeat):
                for g in range(ng):
                    eng = nc.sync if g % 2 == 0 else nc.scalar
                    xg = xtpool.tile([SUB, lb, NC6, BLK], f16)
                    eng.dma_start(
                        xg, xm_d[g].rearrange("k (l c b) -> k l c b",
                                              l=lb, c=NC6)
                    )
                    if stage < 1:
                        continue
                    for l in range(lb):
                        t = g * lb + l
                        hps = ps_h.tile([D_HID, BLK], f32)
                        for c in range(NC6):
                            nc.tensor.matmul(
                                hps, w_sb[:, c, :], xg[:, l, c, :],
                                start=(c == 0), stop=False,
                            )
                        nc.tensor.matmul(
                            hps, w6_sb, x6_sb[:, t * BLK : (t + 1) * BLK],
                            start=False, stop=True,
                        )
                        if stage < 2:
                            continue
                        # bias+relu split DVE/ACT so the halves run in parallel
                        h_sb = hpool.tile([D_HID, BLK], f16)
                        nc.vector.tensor_scalar(
                            h_sb[:, :HB], hps[:, :HB], b1_sb, 0.0,
                            mybir.AluOpType.add, mybir.AluOpType.max,
                        )
                        nc.scalar.activation(
                            h_sb[:, HB:], hps[:, HB:],
                            mybir.ActivationFunctionType.Relu, bias=b1_sb,
                        )
                        if stage < 3:
                            continue
                        # fc2 transposed: out^T[10,512] = w2.T @ h^T; with
                        # defer=True it's emitted after the NEXT block's fc1 so
                        # the in-order PE queue never waits on the relu halves
                        if defer:
                            if pend is not None:
                                fc2_store(*pend)
                            pend = (t, h_sb, g)
                        else:
                            fc2_store(t, h_sb, g)
            if pend is not None:
                fc2_store(*pend)
            if o_dummy is not None:
                nc.sync.dma_start(out_d[0], o_dummy)

    nc.compile()
    return nc


def _get_compiled():
    global _compiled
    if _compiled is None:
        _compiled = _build_bass(mode=MODE)
    return _compiled


def _make_in_maps(x, conv_w, w1, b1, w2, b2):
    w_eff = _build_weff(conv_w, w1)
    w2p = np.zeros((D_HID, D_OUT_PAD), dtype=np.float32)
    w2p[:, :D_OUT] = _round_tf32(w2.astype(np.float32))
    b2p = np.zeros(D_OUT_PAD, dtype=np.float32)
    b2p[:D_OUT] = b2
    b2t = np.tile(b2p, NSUB)
    b1f = np.asarray(b1, dtype=np.float32)

    xs = np.asarray(x, dtype=np.float32).reshape(N_CORES, B_CORE, D_IN)
    maps = []
    for i in range(N_CORES):
        if MODE == "h16":
            x16 = xs[i].astype(np.float16)
            xm = np.ascontiguousarray(
                x16[:, : NC6 * KC6]
                .reshape(NG16, LB16, BLK, NC6, KC6)
                .transpose(0, 4, 1, 3, 2)
            ).reshape(NG16, KC6, LB16 * NC6 * BLK)
            m = {
                "xm": xm,
                "x6": np.ascontiguousarray(x16[:, NC6 * KC6 :].T),
                "w": np.ascontiguousarray(
                    w_eff[: NC6 * KC6].reshape(NC6, KC6, D_HID)
                ).astype(np.float16),
                "w6": w_eff[NC6 * KC6 :].astype(np.float16),
                "b1": b1f,
                "w2": np.asarray(w2, dtype=np.float16),
            }
        elif MODE == "host_t":
            m = {
                "xt": np.ascontiguousarray(xs[i].T),
                "w": np.ascontiguousarray(
                    w_eff[: NC6 * KC6].reshape(NC6, KC6, D_HID)
                ),
                "w6": np.ascontiguousarray(w_eff[NC6 * KC6 :]),
                "b1": b1f,
                "w2": _round_tf32(np.asarray(w2, dtype=np.float32)),
                "b2": np.asarray(b2, dtype=np.float32),
            }
        else:
            m = {
                "x": np.ascontiguousarray(xs[i]),
                "w": w_eff.reshape(NCHUNK, KC, D_HID),
                "b1": b1f,
                "w2": w2p,
                "b2": b2t,
                "idn": np.eye(SUB, dtype=np.float32),
            }
        maps.append(m)
    return maps


def kernel(x, conv_w, w1, b1, w2, b2, **run_kwargs):
    nc = _get_compiled()
    in_maps = _make_in_maps(x, conv_w, w1, b1, w2, b2)
    res = run_bass_kernel_spmd(nc, in_maps, core_ids=list(range(N_CORES)),
                               **run_kwargs)
    if MODE == "h16":
        # device layout [NBLK, 10, 512] fp16 -> [B_CORE, 10] fp32 (+ b2)
        b2f = np.asarray(b2, dtype=np.float32)
        parts = [
            r["out"].transpose(0, 2, 1).reshape(B_CORE, D_OUT)
            .astype(np.float32) + b2f[None, :]
            for r in res.results
        ]
    elif MODE == "host_t":
        # device layout [NBLK, 10, 512] -> [B_CORE, 10]
        parts = [
            r["out"].transpose(0, 2, 1).reshape(B_CORE, D_OUT)
            for r in res.results
        ]
    else:
        parts = [r["out"][:, :D_OUT] for r in res.results]
    out = np.concatenate(parts, axis=0)
    if run_kwargs:
        return out, res
    return out



def _build_bass_host_t(xt_bufs=5, h_bufs=8, o_bufs=8, ph_bufs=4, po_bufs=3,
                       repeat=1, fc2_mode="transposed", defer_fc2=False,
                       load2=True, load_blocks=None):
    """Variant that receives x already feature-major ([784, 8192] per core):
    no on-device transposes; fc1 streams xT tiles straight from HBM.

    Output is written in the on-chip-natural layout ([NBLK, 10, 512] for the
    transposed fc2, [NBLK, 128, NSUB, 16] for batch-major fc2) with contiguous
    stores; the host unscrambles to [8192, 10] in the gather step."""
    nc = bacc.Bacc("TRN2", target_bir_lowering=False, debug=False, num_devices=1)
    f32 = mybir.dt.float32
    f32r = mybir.dt.float32r
    HB = BLK // 2  # fc1/relu column half

    xt_d = nc.dram_tensor("xt", [D_IN, B_CORE], f32r, kind="ExternalInput").ap()
    w_d = nc.dram_tensor("w", [NC6, KC6, D_HID], f32r, kind="ExternalInput").ap()
    w6_d = nc.dram_tensor("w6", [KTAIL, D_HID], f32r, kind="ExternalInput").ap()
    b1_d = nc.dram_tensor("b1", [D_HID], f32, kind="ExternalInput").ap()
    w2_d = nc.dram_tensor("w2", [D_HID, D_OUT], f32r, kind="ExternalInput").ap()
    b2_d = nc.dram_tensor("b2", [D_OUT], f32, kind="ExternalInput").ap()
    if fc2_mode == "transposed":
        out_d = nc.dram_tensor("out", [NBLK, D_OUT, BLK], f32,
                               kind="ExternalOutput").ap()
    else:
        out_d = nc.dram_tensor("out", [NBLK, SUB, NSUB, D_OUT_PAD], f32,
                               kind="ExternalOutput").ap()

    xt_main = xt_d[0 : NC6 * KC6, :].rearrange("(c k) b -> k c b", k=KC6)
    xt_tail = xt_d[NC6 * KC6 :, :]

    with tile.TileContext(nc) as tc:
        with (
            tc.tile_pool(name="const", bufs=1) as const_pool,
            tc.tile_pool(name="xt", bufs=xt_bufs) as xtpool,
            tc.tile_pool(name="h", bufs=h_bufs) as hpool,
            tc.tile_pool(name="o", bufs=o_bufs) as opool,
            tc.tile_pool(name="ph", bufs=ph_bufs, space="PSUM") as ps_h,
            tc.tile_pool(name="po", bufs=po_bufs, space="PSUM") as ps_o,
        ):
            # constants ride SWDGE (gpsimd); both HWDGE rings are reserved
            # for the x stream
            w_sb = const_pool.tile([KC6, NC6, D_HID], f32r)
            nc.gpsimd.dma_start(w_sb, w_d.rearrange("c k h -> k c h"))
            w6_sb = const_pool.tile([KTAIL, D_HID], f32r)
            nc.gpsimd.dma_start(w6_sb, w6_d)
            b1_sb = const_pool.tile([D_HID, 1], f32)
            nc.gpsimd.dma_start(b1_sb, b1_d.rearrange("(h o) -> h o", o=1))
            if fc2_mode == "transposed":
                w2_sb = const_pool.tile([D_HID, D_OUT], f32r)
                nc.gpsimd.dma_start(w2_sb, w2_d)
                b2_sb = const_pool.tile([D_OUT, 1], f32)
                nc.gpsimd.dma_start(b2_sb, b2_d.rearrange("(c o) -> c o", o=1))
            else:
                # cols 10..15 stay garbage; host strips them
                w2_sb = const_pool.tile([D_HID, D_OUT_PAD], f32r)
                nc.gpsimd.dma_start(w2_sb[:, :D_OUT], w2_d)
                b2_sb = const_pool.tile([SUB, NSUB, D_OUT_PAD], f32)
                b2_bcast = bass.AP(
                    tensor=b2_d.tensor, offset=b2_d.offset,
                    ap=[[0, SUB], [0, NSUB]] + list(b2_d.ap),
                )
                nc.gpsimd.dma_start(b2_sb[:, :, :D_OUT], b2_bcast)
            # the 16-row feature tail for ALL blocks in one upfront DMA
            xt6_all = const_pool.tile([KTAIL, B_CORE], f32r)
            nc.gpsimd.dma_start(xt6_all, xt_tail)

            hs = {}

            def fc2_store(t):
                h_sb = hs.pop(t)
                seng = nc.scalar if t % 2 == 0 else nc.sync
                if fc2_mode == "transposed":
                    # fc2 transposed: out^T[10, 512] = w2.T @ h^T -- one
                    # matmul with a 10-column stationary (near-zero LDW);
                    # host untransposes the [NBLK, 10, 512] output
                    ops = ps_o.tile([D_OUT, BLK], f32)
                    nc.tensor.matmul(ops, w2_sb, h_sb, start=True, stop=True)
                    o_sb = opool.tile([D_OUT, BLK], f32)
                    nc.vector.tensor_scalar(
                        o_sb, ops, b2_sb, None, mybir.AluOpType.add
                    )
                    seng.dma_start(out_d[t], o_sb)
                else:
                    ops = ps_o.tile([SUB, NSUB, D_OUT_PAD], f32)
                    for s in range(NSUB):
                        nc.tensor.matmul(
                            ops[:, s, :],
                            h_sb[:, s * SUB : (s + 1) * SUB],
                            w2_sb, start=True, stop=True,
                        )
                    o_sb = opool.tile([SUB, NSUB, D_OUT_PAD], f32)
                    nc.vector.tensor_add(o_sb, ops, b2_sb)
                    seng.dma_start(out_d[t], o_sb)

            xt2s = {}

            for r in range(repeat):
                for t in range(NBLK):
                    # alternate the two HWDGE rings (SP / ACT) per block
                    eng = nc.sync if t % 2 == 0 else nc.scalar
                    LB = load_blocks or (2 if load2 else 1)
                    if LB > 1:
                        # one DMA covers LB blocks: longer contiguous runs
                        # and 1/LB the per-ring issue count
                        if t % LB == 0:
                            xt2 = xtpool.tile([KC6, NC6, LB * BLK], f32r)
                            if r == 0 and t == 0:
                                # chunk-granular first block so fc1(0) starts
                                # ~1 us in instead of after the full group
                                for c in range(NC6):
                                    eng.dma_start(
                                        xt2[:, c, :BLK],
                                        xt_main[:, c, 0:BLK],
                                    )
                                eng.dma_start(
                                    xt2[:, :, BLK : LB * BLK],
                                    xt_main[:, :, BLK : LB * BLK],
                                )
                            else:
                                eng.dma_start(
                                    xt2,
                                    xt_main[:, :, t * BLK : (t + LB) * BLK],
                                )
                            xt2s[t] = xt2
                        xt2 = xt2s[t - (t % LB)]
                        base = (t % LB) * BLK
                        xt_sb = xt2[:, :, base : base + BLK]
                    else:
                        xt_sb = xtpool.tile([KC6, NC6, BLK], f32r)
                        if r == 0 and t == 0:
                            # chunk-granular so the first fc1 starts ~1 us in
                            for c in range(NC6):
                                eng.dma_start(
                                    xt_sb[:, c, :],
                                    xt_main[:, c, t * BLK : (t + 1) * BLK],
                                )
                        else:
                            eng.dma_start(
                                xt_sb, xt_main[:, :, t * BLK : (t + 1) * BLK]
                            )
                    xt6_sb = xt6_all[:, t * BLK : (t + 1) * BLK]

                    # fc1 at full N=512 (weight loads are the serial cost on
                    # PE -- keep matmul count minimal); relu+bias on DVE
                    hps = ps_h.tile([D_HID, BLK], f32)
                    h_sb = hpool.tile([D_HID, BLK], f32r)
                    for c in range(NC6):
                        nc.tensor.matmul(
                            hps, w_sb[:, c, :], xt_sb[:, c, :],
                            start=(c == 0), stop=False,
                        )
                    nc.tensor.matmul(hps, w6_sb, xt6_sb, start=False, stop=True)
                    # bias+relu split across DVE and ACT so the two halves
                    # run in parallel off the fc1->fc2 critical path
                    nc.vector.tensor_scalar(
                        h_sb[:, :HB], hps[:, :HB], b1_sb, 0.0,
                        mybir.AluOpType.add, mybir.AluOpType.max,
                    )
                    nc.scalar.activation(
                        h_sb[:, HB:], hps[:, HB:],
                        mybir.ActivationFunctionType.Relu, bias=b1_sb,
                    )
                    hs[t] = h_sb

                    if defer_fc2:
                        # emit fc2(t-1) after fc1(t): by then relu(t-1) is
                        # long done, so the in-order PE queue never stalls
                        # waiting on the relu halves
                        if t > 0:
                            fc2_store(t - 1)
                        if t == NBLK - 1:
                            fc2_store(t)
                    else:
                        fc2_store(t)

    nc.compile()
    return nc


def _build_bass_dmaonly(repeat=1, nbufs=6, ring2=True, stage=0):
    """Incremental kernel for HW stage-cost bisection.

    stage 0: loads only; 1: +fc1; 2: +relu; 3: +fc2/add; 4: +stores."""
    nc = bacc.Bacc("TRN2", target_bir_lowering=False, debug=False, num_devices=1)
    f32 = mybir.dt.float32
    f32r = mybir.dt.float32r
    xt_d = nc.dram_tensor("xt", [D_IN, B_CORE], f32r, kind="ExternalInput").ap()
    w_d = nc.dram_tensor("w", [NC6, KC6, D_HID], f32r, kind="ExternalInput").ap()
    w6_d = nc.dram_tensor("w6", [KTAIL, D_HID], f32r, kind="ExternalInput").ap()
    b1_d = nc.dram_tensor("b1", [D_HID], f32, kind="ExternalInput").ap()
    w2_d = nc.dram_tensor("w2", [D_HID, D_OUT_PAD], f32r, kind="ExternalInput").ap()
    b2_d = nc.dram_tensor("b2", [NSUB * D_OUT_PAD], f32, kind="ExternalInput").ap()
    out_d = nc.dram_tensor("out", [B_CORE, D_OUT_PAD], f32,
                           kind="ExternalOutput").ap()
    out_v = out_d.rearrange("(t s p) c -> t p s c", s=NSUB, p=SUB)
    xt_main = xt_d[0 : NC6 * KC6, :].rearrange("(c k) b -> k c b", k=KC6)
    xt_tail = xt_d[NC6 * KC6 :, :]
    with tile.TileContext(nc) as tc:
        with (
            tc.tile_pool(name="const", bufs=1) as const_pool,
            tc.tile_pool(name="xt", bufs=nbufs) as xtpool,
            tc.tile_pool(name="h", bufs=4) as hpool,
            tc.tile_pool(name="o", bufs=4) as opool,
            tc.tile_pool(name="ph", bufs=2, space="PSUM") as ps_h,
            tc.tile_pool(name="po", bufs=2, space="PSUM") as ps_o,
        ):
            w_sb = const_pool.tile([KC6, NC6, D_HID], f32r)
            nc.gpsimd.dma_start(w_sb, w_d.rearrange("c k h -> k c h"))
            w6_sb = const_pool.tile([KTAIL, D_HID], f32r)
            nc.gpsimd.dma_start(w6_sb, w6_d)
            b1_sb = const_pool.tile([D_HID, 1], f32)
            nc.gpsimd.dma_start(b1_sb, b1_d.rearrange("(h o) -> h o", o=1))
            w2_sb = const_pool.tile([D_HID, D_OUT_PAD], f32r)
            nc.gpsimd.dma_start(w2_sb, w2_d)
            b2_sb = const_pool.tile([SUB, NSUB * D_OUT_PAD], f32)
            b2_bcast = bass.AP(
                tensor=b2_d.tensor, offset=b2_d.offset,
                ap=[[0, SUB]] + list(b2_d.ap),
            )
            nc.gpsimd.dma_start(b2_sb, b2_bcast)
            xt6_all = const_pool.tile([KTAIL, B_CORE], f32r)
            nc.gpsimd.dma_start(xt6_all, xt_tail)
            o_dummy = const_pool.tile([SUB, D_OUT_PAD], f32)
            nc.gpsimd.memset(o_dummy, 0.0)

            for _ in range(repeat):
                for t in range(NBLK):
                    eng = nc.sync if (t % 2 == 0 or not ring2) else nc.scalar
                    xt_sb = xtpool.tile([KC6, NC6, BLK], f32r)
                    eng.dma_start(
                        xt_sb, xt_main[:, :, t * BLK : (t + 1) * BLK]
                    )
                    if stage < 1:
                        continue
                    hps = ps_h.tile([D_HID, BLK], f32)
                    for c in range(NC6):
                        nc.tensor.matmul(
                            hps, w_sb[:, c, :], xt_sb[:, c, :],
                            start=(c == 0), stop=False,
                        )
                    nc.tensor.matmul(
                        hps, w6_sb, xt6_all[:, t * BLK : (t + 1) * BLK],
                        start=False, stop=True,
                    )
                    if stage < 2:
                        continue
                    h_sb = hpool.tile([D_HID, BLK], f32r)
                    nc.vector.tensor_scalar(
                        h_sb, hps, b1_sb, 0.0,
                        mybir.AluOpType.add, mybir.AluOpType.max,
                    )
                    if stage < 3:
                        continue
                    ops = ps_o.tile([SUB, NSUB, D_OUT_PAD], f32)
                    for s in range(NSUB):
                        nc.tensor.matmul(
                            ops[:, s, :],
                            h_sb[:, s * SUB : (s + 1) * SUB],
                            w2_sb, start=True, stop=True,
                        )
                    o_sb = opool.tile([SUB, NSUB, D_OUT_PAD], f32)
                    nc.vector.tensor_add(
                        o_sb, ops, b2_sb.rearrange("p (s c) -> p s c", s=NSUB)
                    )
                    if stage < 4:
                        continue
                    seng = nc.scalar if t % 2 == 0 else nc.sync
                    seng.dma_start(out_v[t], o_sb)
            nc.sync.dma_start(out_d[0:SUB, :], o_dummy)
    nc.compile()
    return nc

